# revision 8
# baseline (speedup 1.0000x reference)
"""GPT forward (B=4,T=1024,D=1024,H=16,L=8,V=4096) on 8 trn2 NeuronCores, v2.

Sharding: core pair (2b, 2b+1) owns batch element b; even core owns tokens
0..511, odd core 512..1023. Per layer, ONE AllGather of the ln1 output h
(bf16) replaces the K/V exchanges: both cores then compute K/V for the full
sequence locally. kT/vaug are kept in LOCAL-FIRST key order (cols 0..511 =
own tokens, 512..1023 = pair-rank0's tokens); causal masks are per-core
inputs in that coordinate system (even core: tri | zeros, odd: tri | ones).

Layouts: residual xT = [D on partitions, tok free] f32r; h/g1/y activations
bf16 pair-tiles [128, 2, 512]; weights host-packed bf16 panels so device
DMA is contiguous. V and the output heads use activation-stationary matmuls
(out = [tok partitions, feature free]) so no PE transposes are needed.
Optional fp8e4m3 + DoubleRow per site (QKV/proj, MLP) via flags.
"""

import sys

sys.path.insert(0, "/opt/trn_rl_repo")

import numpy as np
import ml_dtypes

import concourse.bass as bass
import concourse.mybir as mybir
import concourse.tile as tile
from concourse import bacc
from concourse.bass_utils import run_bass_kernel_spmd
from concourse.masks import make_identity

P = 128
B, T, D, H, L, V = 4, 1024, 1024, 16, 8, 4096
HD = D // H  # 64
FF = 4 * D
NTOK = T // 2  # 512 tokens per core
TT = NTOK // P  # 4 local token tiles
FT = 2 * TT  # 8 full-seq token tiles
DP = D // P  # 8
FFP = FF // P  # 32
VP = V // P
N_CORES = 8
PAIRS = [[0, 1], [2, 3], [4, 5], [6, 7]]

f32 = mybir.dt.float32
f32r = mybir.dt.float32r
bf16 = mybir.dt.bfloat16
f8 = mybir.dt.float8e4
i32 = mybir.dt.int32
AF = mybir.ActivationFunctionType
ALU = mybir.AluOpType
DR = mybir.MatmulPerfMode.DoubleRow

# fp8 DoubleRow per site (weights*64 fp8, activations fp8, out rescaled)
FP8_QKVP = False  # Wq/Wk/Wv/Wproj matmuls
FP8_MLP = False   # W1/W2 matmuls
WSCALE = 64.0

_CACHE = {}


def _build(n_layers=L):
    nc = bacc.Bacc("TRN2", target_bir_lowering=False, debug=False,
                   num_devices=N_CORES)
    qk_dt = f8 if FP8_QKVP else bf16
    ml_dt = f8 if FP8_MLP else bf16

    d = {}
    # stationary weight panels, PAIRED on nout: [L, nout//2, P, 2, nin(,2), P]
    for name, nout, nin, dt in (("wq_p", DP, DP, qk_dt), ("wk_p", DP, DP, qk_dt),
                                ("wp_p", DP, DP, qk_dt), ("w1_p", FFP, DP, ml_dt),
                                ("w2_p", DP, FFP, ml_dt)):
        shape = ([L, nout // 2, P, 2, nin // 2, 2, P] if dt == f8
                 else [L, nout // 2, P, 2, nin, P])
        d[name] = nc.dram_tensor(name, shape, dt, kind="ExternalInput")
    # moving-weight layouts [L, nin(,2 interleaved), P, Dout]
    d["wv_m"] = nc.dram_tensor(
        "wv_m", [L, DP // 2, P, 2, D] if qk_dt == f8 else [L, DP, P, D],
        qk_dt, kind="ExternalInput")
    for name in ("wl_m", "wd_m"):  # paired on kk: [DP//2, P, 2, V]
        d[name] = nc.dram_tensor(name, [DP // 2, P, 2, V], bf16,
                                 kind="ExternalInput")

    for name in ("bq", "bk", "bproj", "b2"):
        d[name] = nc.dram_tensor(name, [L, D], f32, kind="ExternalInput")
    d["b1"] = nc.dram_tensor("b1", [L, FF], f32, kind="ExternalInput")
    d["bv_row"] = nc.dram_tensor("bv_row", [L, D], qk_dt, kind="ExternalInput")
    d["hl_row"] = nc.dram_tensor("hl_row", [1, V], bf16,
                                 kind="ExternalInput")
    d["hd_row"] = nc.dram_tensor("hd_row", [1, V], bf16,
                                 kind="ExternalInput")
    d["tok_emb"] = nc.dram_tensor("tok_emb", [V, D], f32, kind="ExternalInput")
    d["pos_half"] = nc.dram_tensor("pos_half", [NTOK, D], f32,
                                   kind="ExternalInput")
    d["idx_i32"] = nc.dram_tensor("idx_i32", [NTOK, 1], i32,
                                  kind="ExternalInput")
    d["ltri"] = nc.dram_tensor("ltri", [P, P], bf16, kind="ExternalInput")
    d["rmask"] = nc.dram_tensor("rmask", [P, 1], f32, kind="ExternalInput")

    d["logits"] = nc.dram_tensor("logits", [NTOK, V], f32,
                                 kind="ExternalOutput")
    d["dev"] = nc.dram_tensor("dev", [NTOK, V], f32, kind="ExternalOutput")

    with tile.TileContext(nc) as tc:
        _emit(nc, tc, d, n_layers)
    nc.finalize()
    return nc


def _emit(nc, tc, d, n_layers):
    import contextlib
    ctx = contextlib.ExitStack()
    pers = ctx.enter_context(tc.tile_pool(name="pers", bufs=1))
    wpool = ctx.enter_context(tc.tile_pool(name="wpool", bufs=3))
    vmpool = ctx.enter_context(tc.tile_pool(name="vmpool", bufs=1))
    spool = ctx.enter_context(tc.tile_pool(name="spool", bufs=4))
    dram = ctx.enter_context(tc.tile_pool(name="dram", bufs=2, space="DRAM"))
    pmm = ctx.enter_context(tc.tile_pool(name="pmm", bufs=2, space="PSUM"))
    psc = ctx.enter_context(tc.tile_pool(name="psc", bufs=2, space="PSUM"))
    pya = ctx.enter_context(tc.tile_pool(name="pya", bufs=2, space="PSUM"))

    qk_dt = f8 if FP8_QKVP else bf16
    ml_dt = f8 if FP8_MLP else bf16
    s_qkv = (1.0 / WSCALE) if FP8_QKVP else 1.0
    s_mlp = (1.0 / WSCALE) if FP8_MLP else 1.0

    # ---- constants / persistent state ----
    ident = pers.tile([P, P], f32, name="ident")
    make_identity(nc, ident)
    identb = pers.tile([P, P], bf16, name="identb")
    nc.vector.tensor_copy(identb[:], ident[:])
    ones_f = pers.tile([P, 1], f32, name="ones_f")
    nc.any.memset(ones_f[:], 1.0)
    ones128 = pers.tile([P, 1], f32r, name="ones128")
    nc.vector.tensor_copy(ones128[:], ones_f[:])
    ones1_f = pers.tile([1, P], f32, name="ones1_f")
    nc.any.memset(ones1_f[:], 1.0)
    ones1q = pers.tile([1, P], qk_dt, name="ones1q")
    nc.vector.tensor_copy(ones1q[:], ones1_f[:])
    eps_p = pers.tile([P, 1], f32, name="eps_p")
    nc.any.memset(eps_p[:], 1e-5)
    xTp = [pers.tile([P, 2, NTOK], f32r, name=f"xTp{m}")
           for m in range(DP // 2)]
    xT = [xTp[m // 2][:, m % 2, :] for m in range(DP)]
    # ---- embedding gather + transpose into xT ----
    for t in range(TT):
        ix = spool.tile([P, 1], i32, name=f"ix{t}", tag="ix", bufs=2)
        nc.sync.dma_start(ix[:], d["idx_i32"][t * P:(t + 1) * P, :])
        xn = spool.tile([P, D], f32, name=f"xn{t}", tag="xn", bufs=1)
        nc.gpsimd.indirect_dma_start(
            out=xn[:], out_offset=None, in_=d["tok_emb"][:, :],
            in_offset=bass.IndirectOffsetOnAxis(ap=ix[:, :1], axis=0))
        pt = spool.tile([P, D], f32, name=f"pt{t}", tag="pt", bufs=1)
        nc.sync.dma_start(pt[:], d["pos_half"][t * P:(t + 1) * P, :])
        nc.vector.tensor_add(xn[:], xn[:], pt[:])
        for m in range(DP):
            tr = pmm.tile([P, P], f32, name=f"etr{t}_{m}", tag="mm")
            nc.tensor.transpose(tr[:], xn[:, m * P:(m + 1) * P], ident[:])
            nc.vector.tensor_copy(xTp[m // 2][:, m % 2, t * P:(t + 1) * P],
                                  tr[:])

    ltri1 = pers.tile([P, P], bf16, name="ltri1")
    nc.sync.dma_start(ltri1[:], d["ltri"][:, :])
    rmask_t = pers.tile([P, 1], f32, name="rmask_t")
    nc.sync.dma_start(rmask_t[:], d["rmask"][:, :])

    # activations as pair-tiles [P, 2, NTOK]
    hB = [pers.tile([P, 2, NTOK], qk_dt, name=f"hB{m}") for m in range(DP // 2)]
    hR = [pers.tile([P, 2, NTOK], qk_dt, name=f"hR{m}") for m in range(DP // 2)]
    yB = [pers.tile([P, 2, NTOK], qk_dt, name=f"yB{m}") for m in range(DP // 2)]
    g1 = [pers.tile([P, 2, NTOK], ml_dt, name=f"g1_{m}")
          for m in range(FFP // 2)]
    qT = [pers.tile([P, NTOK], bf16, name=f"qT{m}") for m in range(DP)]
    kT = [pers.tile([P, T], bf16, name=f"kT{m}") for m in range(DP)]
    vaug = [pers.tile([P, H, HD + 1], bf16, name=f"vaug{t}")
            for t in range(FT)]
    for t in range(FT):
        nc.any.memset(vaug[t][:, :, HD:HD + 1], 1.0)
        if t >= TT:  # remote tiles: ones column zeroed on even cores
            nc.vector.tensor_scalar_mul(vaug[t][:, :, HD:HD + 1],
                                        vaug[t][:, :, HD:HD + 1],
                                        rmask_t[:, 0:1])

    ones1b = pers.tile([1, P], bf16, name="ones1b")
    nc.vector.tensor_copy(ones1b[:], ones1_f[:])
    ones1r = pers.tile([1, P], f32r, name="ones1r")
    nc.vector.tensor_copy(ones1r[:], ones1_f[:])

    def col_tile(src_d, l, nparts, name):
        tl = spool.tile([P, nparts], f32, name=name, tag=name)
        nc.sync.dma_start(tl[:], src_d[l].rearrange("(a p) -> p a", p=P))
        return tl

    def layer_norm(src, dst_pair, tag):
        # src: 8 x f32r [P, NTOK]; dst_slice(c) -> [P, NTOK] AP (any dtype).
        # Stats share one PSUM bank (partitions 0 and 32); mean/rstd are
        # Pool-broadcast to full width so no PE matmul sits in the chain.
        st1 = pmm.tile([1, NTOK], f32, name=f"st1{tag}", tag="mm")
        st2 = pmm.tile([1, NTOK], f32, name=f"st2{tag}", tag="mm")
        for m in range(DP):
            nc.tensor.matmul(st1[:], ones128[:], src[m],
                             start=(m == 0), stop=(m == DP - 1))
        for m in range(DP):
            sq = spool.tile([P, NTOK], f32r, name=f"sq{tag}{m}", tag="sq",
                            bufs=2)
            nc.scalar.activation(sq[:], src[m], AF.Square)
            nc.tensor.matmul(st2[:], ones128[:], sq[:],
                             start=(m == 0), stop=(m == DP - 1))
        mrow = spool.tile([1, NTOK], f32, name=f"mrow{tag}", tag="mrow",
                          bufs=1)
        nc.vector.tensor_scalar_mul(mrow[:], st1[:], 1.0 / D)
        vrow = spool.tile([1, NTOK], f32, name=f"vrow{tag}", tag="vrow",
                          bufs=1)
        nc.vector.tensor_scalar_mul(vrow[:], st2[:], 1.0 / D)
        mb = spool.tile([P, NTOK], f32, name=f"mb{tag}", tag="mb", bufs=1)
        nc.gpsimd.partition_broadcast(mb[:], mrow[:])
        vb = spool.tile([P, NTOK], f32, name=f"vb{tag}", tag="vb", bufs=1)
        nc.gpsimd.partition_broadcast(vb[:], vrow[:])
        msqb = spool.tile([P, NTOK], f32, name=f"msqb{tag}", tag="msqb",
                          bufs=1)
        nc.scalar.activation(msqb[:], mb[:], AF.Square)
        nc.vector.tensor_sub(vb[:], vb[:], msqb[:])
        nc.scalar.activation(vb[:], vb[:], AF.Sqrt, bias=eps_p[:, 0:1])
        rstdb = spool.tile([P, NTOK], f32r, name=f"rstdb{tag}", tag="rstdb",
                          bufs=1)
        with nc.allow_low_precision("f32r is fp32 bits; rounding only at PE"):
            nc.vector.reciprocal(rstdb[:], vb[:])
        for mp in range(DP // 2):
            dp = dst_pair(mp)
            nc.vector.tensor_sub(dp, xTp[mp][:],
                                 mb[:, None, :].broadcast_to([P, 2, NTOK]))
            nc.vector.tensor_mul(dp, dp,
                                 rstdb[:, None, :].broadcast_to([P, 2,
                                                                 NTOK]))

    def mm_accum(ps, panel, moving, nin, is_f8, first=True, last=True,
                 kk_base=0):
        # accumulate ps += sum_kk panel[kk].T @ moving[kk_base + kk]
        skip = not first
        if is_f8:
            for j in range(nin // 2):
                nc.tensor.matmul(ps[:], panel[:, j, :, :],
                                 moving(kk_base // 2 + j, None),
                                 start=(j == 0 and first),
                                 stop=(j == nin // 2 - 1 and last),
                                 perf_mode=DR, skip_group_check=skip)
        else:
            for kk in range(nin):
                kg = kk_base + kk
                nc.tensor.matmul(ps[:], panel[:, kk, :],
                                 moving(kg // 2, kg % 2),
                                 start=(kk == 0 and first),
                                 stop=(kk == nin - 1 and last),
                                 skip_group_check=skip)

    def load_pair(w_d, l, mp, nin, dt, kk0=0, name=""):
        # one DMA brings the panels for outputs 2*mp and 2*mp+1
        src = (w_d[l, mp, :, :, kk0 // 2:(kk0 + nin) // 2] if dt == f8
               else w_d[l, mp, :, :, kk0:kk0 + nin])
        shape = [P, 2, nin // 2, 2, P] if dt == f8 else [P, 2, nin, P]
        pan = wpool.tile(shape, dt, name=f"pan{name}{mp}_{kk0}", tag="wpan")
        nc.sync.dma_start(pan[:], src)
        return pan

    hb_mv = (lambda j, s: (hB[j][:, :, :] if s is None else hB[j][:, s, :]))
    hr_mv = (lambda j, s: (hR[j][:, :, :] if s is None else hR[j][:, s, :]))
    yb_mv = (lambda j, s: (yB[j][:, :, :] if s is None else yB[j][:, s, :]))
    g1_mv = (lambda j, s: (g1[j][:, :, :] if s is None else g1[j][:, s, :]))

    for l in range(n_layers):
        layer_norm(xT, lambda mp: hB[mp][:], f"a{l}")

        # ---- AllGather h (one collective per layer) ----
        hb_in = dram.tile([DP // 2, P, 2, NTOK], qk_dt, name=f"hbin{l}",
                          tag="hbin")
        hb_out = dram.tile([2, DP // 2, P, 2, NTOK], qk_dt, name=f"hbout{l}",
                           tag="hbout")
        for i in range(DP // 2):
            nc.sync.dma_start(hb_in[i], hB[i][:])
        nc.gpsimd.collective_compute(
            "AllGather", ALU.bypass, replica_groups=PAIRS,
            ins=[hb_in[:].opt()], outs=[hb_out[:].opt()])

        # ---- Q (local tokens only) ----
        bqc = col_tile(d["bq"], l, DP, "bqc")
        for mp in range(DP // 2):
            pan = load_pair(d["wq_p"], l, mp, DP, qk_dt, name="q")
            for s in range(2):
                m = 2 * mp + s
                ps = pmm.tile([P, NTOK], f32, name=f"mmq{m}", tag="mm")
                mm_accum(ps, pan[:, s], hb_mv, DP, qk_dt == f8)
                nc.vector.tensor_scalar(qT[m][:], ps[:], s_qkv,
                                        bqc[:, m:m + 1], ALU.mult, ALU.add)

        # ---- K/V local halves first, so phase-A attention can overlap
        # the h AllGather; remote halves are computed after it lands ----
        bkc = col_tile(d["bk"], l, DP, "bkc")

        def k_half(half, mv, pre=None):
            for mp in range(DP // 2):
                pan = pre[mp] if pre is not None and mp < len(pre) else \
                    load_pair(d["wk_p"], l, mp, DP, qk_dt,
                              name=("kl", "kr")[half])
                for s in range(2):
                    m = 2 * mp + s
                    ps = pmm.tile([P, NTOK], f32, name=f"mmk{m}_{half}",
                                  tag="mm")
                    mm_accum(ps, pan[:, s], mv, DP, qk_dt == f8)
                    nc.vector.tensor_scalar(
                        kT[m][:, half * NTOK:(half + 1) * NTOK], ps[:],
                        s_qkv, bkc[:, m:m + 1], ALU.mult, ALU.add)

        nvm = DP // 2 if qk_dt == f8 else DP
        vm = []
        for kk in range(nvm):
            t_ = vmpool.tile([P, 2, D] if qk_dt == f8 else [P, D], qk_dt,
                             name=f"vm{kk}", tag=f"vm{kk}")
            nc.sync.dma_start(t_[:], d["wv_m"][l, kk])
            vm.append(t_)
        bvr = spool.tile([1, D], qk_dt, name="bvr", tag="bvr", bufs=1)
        nc.sync.dma_start(bvr[:], d["bv_row"][l:l + 1, :])

        def v_tile(t):
            hsrc, tt = (hB, t) if t < TT else (hR, t - TT)
            for half in range(2):
                ps = pmm.tile([P, NTOK], f32, name=f"mmv{t}_{half}",
                              tag="mm")
                nc.tensor.matmul(ps[:], ones1q[:],
                                 bvr[0:1, half * NTOK:(half + 1) * NTOK],
                                 start=True, stop=False)
                if qk_dt == f8:
                    for j in range(DP // 2):
                        nc.tensor.matmul(
                            ps[:], hsrc[j][:, :, tt * P:(tt + 1) * P],
                            vm[j][:, :, half * NTOK:(half + 1) * NTOK],
                            start=False, stop=(j == DP // 2 - 1),
                            perf_mode=DR, skip_group_check=True)
                else:
                    for kk in range(DP):
                        nc.tensor.matmul(
                            ps[:], hsrc[kk // 2][:, kk % 2,
                                                 tt * P:(tt + 1) * P],
                            vm[kk][:, half * NTOK:(half + 1) * NTOK],
                            start=False, stop=(kk == DP - 1),
                            skip_group_check=True)
                vslice = vaug[t][:, half * (H // 2):(half + 1) * (H // 2),
                                 0:HD]
                psv = ps[:].rearrange("p (h f) -> p h f", h=H // 2)
                if t < TT:
                    nc.vector.tensor_scalar_mul(vslice, psv, s_qkv)
                else:  # remote: zeroed on even cores via rmask
                    nc.vector.tensor_scalar(vslice, psv, rmask_t[:, 0:1],
                                            s_qkv, ALU.mult, ALU.mult)

        k_half(0, hb_mv)
        for t in range(TT):
            v_tile(t)

        def attn_pair(m2, ya_pair, j, first, last):
            # heads (2*m2, 2*m2+1) share one paired score/exp tile so the
            # ACT exp amortizes its per-op overhead over both heads.
            # Local tiles only touch queries >= j*P (causality).
            q0 = j * P if j < TT else 0
            sc2 = psc.tile([P, 2, NTOK], f32, name=f"sc{m2}_{j}", tag="sc")
            for s in range(2):
                off = s * HD
                nc.tensor.matmul(sc2[:, s, q0:],
                                 kT[m2][off:off + HD, j * P:(j + 1) * P],
                                 qT[m2][off:off + HD, q0:], start=True,
                                 stop=True, skip_group_check=True)
            ex2 = spool.tile([P, 2, NTOK], bf16, name=f"ex{m2}_{j}",
                             tag="ex", bufs=5)
            nc.scalar.activation(ex2[:, :, q0:], sc2[:, :, q0:], AF.Exp,
                                 scale=1.0 / np.sqrt(HD))
            if j < TT:  # diagonal block: triangular mask (identical
                # [P,P] pattern for every local tile)
                nc.vector.tensor_mul(
                    ex2[:, :, q0:q0 + P], ex2[:, :, q0:q0 + P],
                    ltri1[:, None, :].broadcast_to([P, 2, P]))
            for s in range(2):
                nc.tensor.matmul(ya_pair[s][:, q0:],
                                 vaug[j][:, 2 * m2 + s, :],
                                 ex2[:, s, q0:], start=first, stop=last,
                                 skip_group_check=True)

        # phase A: local keys, spill partial numerator/denominator
        yLs = [spool.tile([HD + 1, NTOK], bf16, name=f"yls{hh}",
                          tag=f"yls{hh}", bufs=1) for hh in range(H)]
        for m2 in range(DP):
            yap = [pya.tile([HD + 1, NTOK], f32, name=f"yal{2 * m2 + s}",
                            tag="ya") for s in range(2)]
            for j in range(TT):
                attn_pair(m2, yap, j, j == 0, j == TT - 1)
            for s in range(2):
                nc.scalar.copy(yLs[2 * m2 + s][:], yap[s][:])  # ACT spill

        # remote K/V once the AllGather has landed. The first K-remote
        # panels are DMA'd BEFORE the hR read-back is enqueued, so they
        # don't queue behind the collective (head-of-line blocking).
        krpre = [load_pair(d["wk_p"], l, mp, DP, qk_dt, name="kr")
                 for mp in range(2)]
        # fence: reading hR (WAR vs the DMA write) AND the last phase-A
        # spill holds the hR fill — and hence every remote-dependent
        # matmul — behind phase A in the in-order PE stream.
        fence = spool.tile([1, 1], f32, name=f"fence{l}", tag="fence",
                           bufs=2)
        nc.vector.tensor_add(fence[:], hR[0][0:1, 0, 0:1],
                             yLs[H - 1][0:1, 0:1])
        for i in range(DP // 2):
            nc.sync.dma_start(hR[i][:], hb_out[0, i])
        k_half(1, hr_mv, pre=krpre)
        for t in range(TT, FT):
            v_tile(t)

        # phase B: remote keys accumulate onto the phase-A partials
        # (PE re-injects the bf16 spill via an identity matmul), then
        # normalize with a row reciprocal broadcast by Pool.
        for m2 in range(DP):
            yap = [pya.tile([HD + 1, NTOK], f32, name=f"yar{2 * m2 + s}",
                            tag="ya") for s in range(2)]
            for s in range(2):
                nc.tensor.matmul(yap[s][:], identb[0:HD + 1, 0:HD + 1],
                                 yLs[2 * m2 + s][:], start=True, stop=False)
            for j in range(TT, FT):
                attn_pair(m2, yap, j, False, j == FT - 1)
            for s in range(2):
                hh = 2 * m2 + s
                dr = spool.tile([1, NTOK], f32r, name=f"dr{hh}", tag="dr",
                                bufs=1)
                nc.vector.tensor_copy(dr[:], yap[s][HD:HD + 1, :])
                with nc.allow_low_precision("f32r is fp32 bits"):
                    nc.vector.reciprocal(dr[:], dr[:])
                ysl = yB[m2 // 2][(hh % 2) * HD:(hh % 2) * HD + HD,
                                  m2 % 2, :]
                if m2 < DP - 2:
                    denb = spool.tile([HD, NTOK], f32r, name=f"denb{hh}",
                                      tag="denb", bufs=2)
                    nc.gpsimd.partition_broadcast(denb[:], dr[:])
                    nc.vector.tensor_mul(ysl, yap[s][0:HD, :], denb[:])
                else:
                    # last heads gate proj: broadcast on PE (in-stream,
                    # pmm is idle here) instead of the slower Pool path
                    rbc = pmm.tile([HD, NTOK], f32, name=f"rbch{hh}",
                                   tag="mm")
                    nc.tensor.matmul(rbc[:], ones1r[0:1, 0:HD], dr[:],
                                     start=True, stop=True)
                    yraw = spool.tile([HD, NTOK], f32r, name=f"yraw{hh}",
                                      tag="denb", bufs=2)
                    nc.vector.tensor_copy(yraw[:], yap[s][0:HD, :])
                    nc.vector.tensor_mul(ysl, yraw[:], rbc[:])

        # ---- proj + residual ----
        bpc = col_tile(d["bproj"], l, DP, "bpc")
        for mp in range(DP // 2):
            pan = load_pair(d["wp_p"], l, mp, DP, qk_dt, name="p")
            for s in range(2):
                m = 2 * mp + s
                ps = pmm.tile([P, NTOK], f32, name=f"mmp{m}", tag="mm")
                mm_accum(ps, pan[:, s], yb_mv, DP, qk_dt == f8)
                assert s_qkv == 1.0
                nc.vector.scalar_tensor_tensor(
                    xT[m], ps[:], bpc[:, m:m + 1], xT[m],
                    ALU.add, ALU.add)

        # ---- LN2 -> h2 (reuse hB) ----
        layer_norm(xT, lambda mp: hB[mp][:], f"b{l}")
        # W1 consumes ml_dt; if dtypes differ, cast into fp8 tiles
        if ml_dt != qk_dt:
            h2c = [spool.tile([P, 2, NTOK], ml_dt, name=f"h2c{m}",
                              tag=f"h2c{m}", bufs=2) for m in range(DP // 2)]
            for m in range(DP // 2):
                nc.vector.tensor_copy(h2c[m][:], hB[m][:])
            h2_mv = (lambda j, s, _h=None: (h2c[j][:, :, :] if s is None
                                            else h2c[j][:, s, :]))
        else:
            h2_mv = hb_mv

        # ---- MLP ----
        b1c = spool.tile([P, FFP], f32, name="b1c", tag="b1c")
        nc.sync.dma_start(b1c[:], d["b1"][l].rearrange("(a p) -> p a", p=P))
        for mp in range(FFP // 2):
            pan = load_pair(d["w1_p"], l, mp, DP, ml_dt, name="f")
            for s in range(2):
                m = 2 * mp + s
                ps = pmm.tile([P, NTOK], f32, name=f"mmf1{m}", tag="mm")
                mm_accum(ps, pan[:, s], h2_mv, DP, ml_dt == f8)
                nc.scalar.activation(g1[mp][:, s, :], ps[:], AF.Gelu,
                                     bias=b1c[:, m:m + 1], scale=s_mlp)
        b2c = col_tile(d["b2"], l, DP, "b2c")
        NQ = FFP // 4  # kk per quarter-load
        for mp in range(DP // 2):
            pss = [pmm.tile([P, NTOK], f32, name=f"mmf2{2 * mp + s}",
                            tag="mm") for s in range(2)]
            for q4 in range(4):  # quartered kk to bound the wpan slot size
                pan = load_pair(d["w2_p"], l, mp, NQ, ml_dt, kk0=q4 * NQ,
                                name="g" + "abcd"[q4])
                for s in range(2):
                    mm_accum(pss[s], pan[:, s], g1_mv, NQ, ml_dt == f8,
                             first=(q4 == 0), last=(q4 == 3),
                             kk_base=q4 * NQ)
            for s in range(2):
                m = 2 * mp + s
                assert s_mlp == 1.0
                nc.vector.scalar_tensor_tensor(
                    xT[m], pss[s][:], b2c[:, m:m + 1], xT[m],
                    ALU.add, ALU.add)

    # ---- final LN + heads (activation-stationary, no transposes) ----
    # write into qT (always bf16) so head precision is dtype-flag-independent
    # prefetch the first head-weight column block before the LN so the
    # heads' first matmuls aren't DMA-gated
    wlc_pre = []
    for kp in range(DP // 2):
        t_ = vmpool.tile([P, 2, NTOK], bf16, name=f"wlclg0_{kp}",
                         tag=f"vm{kp}")
        nc.sync.dma_start(t_[:], d["wl_m"][kp, :, :, 0:NTOK])
        wlc_pre.append(t_)
    layer_norm(xT, lambda mp: hB[mp][:], "f")
    for m in range(DP):
        nc.vector.tensor_copy(qT[m][:], hB[m // 2][:, m % 2, :])
    for w_d, out_d, brow_d, tg in ((d["wl_m"], d["logits"], d["hl_row"],
                                    "lg"),
                                   (d["wd_m"], d["dev"], d["hd_row"],
                                    "dv")):
        for vc in range(V // NTOK):
            hbrow = spool.tile([1, NTOK], bf16, name=f"hbrow{tg}{vc}",
                               tag="hbrow", bufs=1)
            nc.sync.dma_start(hbrow[:],
                              brow_d[:, vc * NTOK:(vc + 1) * NTOK])
            if tg == "lg" and vc == 0:
                wlc = wlc_pre
            else:
                wlc = []
                for kp in range(DP // 2):
                    t_ = vmpool.tile([P, 2, NTOK], bf16,
                                     name=f"wlc{tg}{vc}_{kp}",
                                     tag=f"vm{kp}")
                    nc.sync.dma_start(
                        t_[:], w_d[kp, :, :, vc * NTOK:(vc + 1) * NTOK])
                    wlc.append(t_)
            for t in range(TT):
                ps = pmm.tile([P, NTOK], f32, name=f"mmh{tg}{vc}_{t}",
                              tag="mm")
                nc.tensor.matmul(ps[:], ones1b[:], hbrow[0:1, :],
                                 start=True, stop=False)
                for kk in range(DP):
                    nc.tensor.matmul(ps[:],
                                     qT[kk][:, t * P:(t + 1) * P],
                                     wlc[kk // 2][:, kk % 2, :],
                                     start=False,
                                     stop=(kk == DP - 1),
                                     skip_group_check=True)
                ot = spool.tile([P, NTOK], f32, name=f"ot{tg}{vc}_{t}",
                                tag="ot", bufs=2)
                nc.vector.tensor_copy(ot[:], ps[:])
                nc.sync.dma_start(
                    out_d[t * P:(t + 1) * P, vc * NTOK:(vc + 1) * NTOK],
                    ot[:])
    ctx.close()


def _pack(W, nout, nin, fp8):
    # W [..., Dout, Din]; returns panels paired on the output dim:
    # [..., nout//2, P, 2, nin(,2), P]
    lead = W.shape[:-2]
    nl = len(lead)
    Wt = np.swapaxes(W, -1, -2)  # [..., Din, Dout]
    Wt = Wt.reshape(*lead, nin, P, nout, P)
    if fp8:
        Wt = Wt.reshape(*lead, nin // 2, 2, P, nout, P)
        arr = np.transpose(Wt, (*range(nl), nl + 3, nl + 2, nl, nl + 1,
                                nl + 4))  # [..., nout, P, nin//2, 2, P]
        arr = arr.reshape(*lead, nout // 2, 2, P, nin // 2, 2, P)
        arr = np.transpose(arr, (*range(nl), nl, nl + 2, nl + 1, nl + 3,
                                 nl + 4, nl + 5))
        arr = (arr * WSCALE).astype(ml_dtypes.float8_e4m3)
    else:
        arr = np.transpose(Wt, (*range(nl), nl + 2, nl + 1, nl, nl + 3))
        arr = arr.reshape(*lead, nout // 2, 2, P, nin, P)
        arr = np.transpose(arr, (*range(nl), nl, nl + 2, nl + 1, nl + 3,
                                 nl + 4))
        arr = arr.astype(ml_dtypes.bfloat16)
    return np.ascontiguousarray(arr)


def _pack_moving(W, fp8, scale=False):
    # W [..., Dout, Din] -> [..., nin(,2), P, Dout]
    lead = W.shape[:-2]
    nl = len(lead)
    dout = W.shape[-2]
    nin = W.shape[-1] // P
    Wt = np.swapaxes(W, -1, -2).reshape(*lead, nin, P, dout)
    if fp8:
        Wt = Wt.reshape(*lead, nin // 2, 2, P, dout)
        arr = np.transpose(Wt, (*range(nl), nl, nl + 2, nl + 1, nl + 3))
        arr = (arr * WSCALE).astype(ml_dtypes.float8_e4m3)
    else:
        arr = Wt.astype(ml_dtypes.bfloat16)
    return np.ascontiguousarray(arr)


def _pack_heads(W):
    # W [V, D] -> [DP//2, P, 2, V] bf16 (kk pairs contiguous per partition)
    arr = W.T.reshape(DP // 2, 2, P, V)
    return np.ascontiguousarray(
        np.transpose(arr, (0, 2, 1, 3)).astype(ml_dtypes.bfloat16))


def _prep_inputs(idx, tok_emb_w, pos_emb, ln1_w, ln1_b, Wq, bq, Wk, bk, Wv,
                 bv, Wproj, bproj, ln2_w, ln2_b, W1, b1, W2, b2, lnf_w,
                 lnf_b, Wlogit, Wdev):
    f = np.ascontiguousarray
    qk8, ml8 = FP8_QKVP, FP8_MLP
    qdt = ml_dtypes.float8_e4m3 if qk8 else ml_dtypes.bfloat16
    # fold the LN affines into the consuming weights (exact algebra):
    #   W @ (xh*w + b) + bias  ==  (W*diag(w)) @ xh + (W @ b + bias)
    Wq_f = Wq * ln1_w[:, None, :]
    Wk_f = Wk * ln1_w[:, None, :]
    Wv_f = Wv * ln1_w[:, None, :]
    W1_f = W1 * ln2_w[:, None, :]
    bq_f = bq + np.einsum("lod,ld->lo", Wq, ln1_b)
    bk_f = bk + np.einsum("lod,ld->lo", Wk, ln1_b)
    bv_f = bv + np.einsum("lod,ld->lo", Wv, ln1_b)
    b1_f = b1 + np.einsum("lod,ld->lo", W1, ln2_b)
    Wl_f = Wlogit * lnf_w[None, :]
    Wd_f = Wdev * lnf_w[None, :]
    hl_row = (Wlogit @ lnf_b)[None, :]
    hd_row = (Wdev @ lnf_b)[None, :]
    com = {
        "wq_p": _pack(Wq_f, DP, DP, qk8), "wk_p": _pack(Wk_f, DP, DP, qk8),
        "wp_p": _pack(Wproj, DP, DP, qk8),
        "w1_p": _pack(W1_f, FFP, DP, ml8), "w2_p": _pack(W2, DP, FFP, ml8),
        "wv_m": _pack_moving(Wv_f, qk8),
        "wl_m": _pack_heads(Wl_f), "wd_m": _pack_heads(Wd_f),
        "hl_row": f(hl_row.astype(ml_dtypes.bfloat16)),
        "hd_row": f(hd_row.astype(ml_dtypes.bfloat16)),
        "bq": bq_f, "bk": bk_f, "bproj": bproj, "b2": b2, "b1": b1_f,
        "bv_row": f((bv_f * (WSCALE if qk8 else 1.0)).astype(qdt)),
        "tok_emb": tok_emb_w,
    }
    # diagonal-block causal mask (same [P,P] pattern for every local
    # k-tile); remote keys are handled by rmask-zeroed vaug
    qi = np.arange(P)[None, :]
    ki = np.arange(P)[:, None]
    com["ltri"] = np.ascontiguousarray((qi >= ki).astype(ml_dtypes.bfloat16))
    in_maps = []
    for c in range(N_CORES):
        b, hf = c // 2, c % 2
        m = dict(com)
        m["idx_i32"] = f(idx[b, hf * NTOK:(hf + 1) * NTOK]
                         .astype(np.int32)[:, None])
        m["pos_half"] = f(pos_emb[0, hf * NTOK:(hf + 1) * NTOK])
        m["rmask"] = np.full((P, 1), float(hf), np.float32)
        in_maps.append(m)
    return in_maps


def kernel(**inputs):
    if "nc" not in _CACHE:
        _CACHE["nc"] = _build()
    nc = _CACHE["nc"]
    in_maps = _prep_inputs(**{k: np.asarray(v) for k, v in inputs.items()})
    res = run_bass_kernel_spmd(nc, in_maps, core_ids=list(range(N_CORES)))
    logits = np.empty((B, T, V), np.float32)
    dev = np.empty((B, T, V), np.float32)
    for c in range(N_CORES):
        b, hf = c // 2, c % 2
        logits[b, hf * NTOK:(hf + 1) * NTOK] = res.results[c]["logits"]
        dev[b, hf * NTOK:(hf + 1) * NTOK] = res.results[c]["dev"]
    return logits, dev
